# revision 9
# baseline (speedup 1.0000x reference)
"""Trainium2 Bass kernel for nn_DAHead_Channel (conv3x3+BN+ReLU -> channel attention -> conv3x3+BN+ReLU).

Data-parallel over batch across 8 NeuronCores (1 batch element each).
Training-mode BatchNorm needs global (N,H,W) stats: conv1 uses one tiny
AllReduce, conv2 fires one AllReduce per output-channel chunk so the collective
latency hides under the next chunk's matmuls.

All matmul operands are bf16 (PSUM accumulation stays fp32): the PE streams
1 row/cycle either way, but the per-matmul LD_WEIGHTS drops from ~224ns (fp32)
to ~half, and input DMA bytes halve. Loop orders keep the stationary operand
constant across consecutive matmuls where stats timing allows.

Hardcoded problem shape: x [8,512,64,64] f32, w1 [128,512,3,3], w2 [512,128,3,3].
"""

import numpy as np
import ml_dtypes

import concourse.bacc as bacc
import concourse.mybir as mybir
import concourse.tile as tile
from concourse import bass_utils

N_CORES = 8
B, CIN, H, W = 8, 512, 64, 64
HP, WP = H + 2, W + 2          # zero-padded spatial dims
PIX = H * W                    # 4096
PPAD = HP * WP                 # 4356
NT = 8                         # HW tiles of 512 pixels (8 rows of 64)
NC1 = CIN // 128               # 4 input-channel chunks for conv1
NC2 = CIN // 128               # 4 output-channel chunks for conv2
NK = PIX // 128                # 32 transpose chunks for attention
EPS = 1e-5
NHW = B * PIX                  # BN count = 32768
F32 = mybir.dt.float32
BF16 = mybir.dt.bfloat16
BF16_NP = ml_dtypes.bfloat16


def _emit(nc):
    xin = nc.dram_tensor("xp", [CIN, PPAD], BF16, kind="ExternalInput")
    w1in = nc.dram_tensor("w1t", [128, NC1 * 9 * 128], BF16, kind="ExternalInput")
    w2in = nc.dram_tensor("w2t", [128, 9 * NC2 * 128], BF16, kind="ExternalInput")
    g1in = nc.dram_tensor("g1b1", [128, 2], F32, kind="ExternalInput")
    g2in = nc.dram_tensor("g2b2", [128, 8], F32, kind="ExternalInput")
    btin = nc.dram_tensor("betav", [128, 1], F32, kind="ExternalInput")
    idin = nc.dram_tensor("ident", [128, 128], BF16, kind="ExternalInput")
    out = nc.dram_tensor("out", [CIN, PIX], F32, kind="ExternalOutput")

    AF = mybir.ActivationFunctionType
    ALU = mybir.AluOpType
    AX = mybir.AxisListType
    inv_n = 1.0 / NHW

    with tile.TileContext(nc) as tc:
        with (
            tc.tile_pool(name="persist", bufs=1) as P,
            tc.tile_pool(name="rot", bufs=4) as R,
            tc.tile_pool(name="ps", bufs=8, space="PSUM") as PS,
            tc.tile_pool(name="dram", bufs=1, space="DRAM") as DR,
        ):
            # ---------------- persistent SBUF ----------------
            w2sb = P.tile([128, 9 * NC2 * 128], BF16, tag="w2")
            h1 = P.tile([128, PIX], F32, tag="h1")
            xf = P.tile([128, PIX], BF16, tag="xf")
            c2in = P.tile([128, PPAD], BF16, tag="c2in")
            ident = P.tile([128, 128], BF16, tag="ident")
            g1b1 = P.tile([128, 2], F32, tag="g1b1")
            g2b2 = P.tile([128, 8], F32, tag="g2b2")
            betav = P.tile([128, 1], F32, tag="betav")
            s1 = P.tile([128, NT], F32, tag="s1")
            q1 = P.tile([128, NT], F32, tag="q1")
            st1 = P.tile([128, 2], F32, tag="st1")
            st1g = P.tile([128, 2], F32, tag="st1g")
            s2 = P.tile([128, NC2 * NT], F32, tag="s2")
            q2 = P.tile([128, NC2 * NT], F32, tag="q2")
            st2 = P.tile([128, 8], F32, tag="st2")     # per-chunk [sum, sumsq]
            st2g = P.tile([128, 8], F32, tag="st2g")
            co1 = P.tile([128, 10], F32, tag="co1")
            co2 = P.tile([128, 32], F32, tag="co2")    # 8 cols per chunk
            att_e = P.tile([128, 128], BF16, tag="att_e")
            attwT = P.tile([128, 128], BF16, tag="attwT")
            asml = P.tile([128, 4], F32, tag="asml")
            tbl = P.tile([128, 1], F32, tag="tbl")     # act-table warm dummy

            nc.vector.memset(c2in[:], 0.0)

            ar0i = DR.tile([128, 1], F32, tag="ar0i")
            ar0o = DR.tile([128, 1], F32, tag="ar0o", addr_space="Shared")
            ar0o2 = DR.tile([128, 1], F32, tag="ar0o2", addr_space="Shared")
            zsc = P.tile([128, 1], F32, tag="zsc")
            zsc2 = P.tile([128, 1], F32, tag="zsc2")

            ar1i = DR.tile([128, 2], F32, tag="ar1i")
            ar1o = DR.tile([128, 2], F32, tag="ar1o", addr_space="Shared")
            ar2i = [DR.tile([128, 2], F32, tag=f"ar2i{c}", name=f"ar2i{c}")
                    for c in range(NC2)]
            ar2o = [DR.tile([128, 2], F32, tag=f"ar2o{c}", name=f"ar2o{c}",
                            addr_space="Shared")
                    for c in range(NC2)]

            # ---------------- phase 1: conv1 + attention ----------------
            with tc.tile_pool(name="phase1", bufs=1) as P1:
                w1sb = P1.tile([128, NC1 * 9 * 128], BF16, tag="w1")
                xsb = [
                    P1.tile([128, PPAD], BF16, tag=f"x{c}", name=f"xsb{c}")
                    for c in range(NC1)
                ]
                # interleave weight-chunk and x-chunk DMAs (split for fast start)
                for c in range(NC1):
                    nc.gpsimd.dma_start(
                        out=w1sb[:, c * 1152 : (c + 1) * 1152],
                        in_=w1in.ap()[:, c * 1152 : (c + 1) * 1152],
                    )
                    half = PPAD // 2
                    nc.sync.dma_start(
                        out=xsb[c][:, 0:half],
                        in_=xin.ap()[c * 128 : (c + 1) * 128, 0:half],
                    )
                    nc.sync.dma_start(
                        out=xsb[c][:, half:PPAD],
                        in_=xin.ap()[c * 128 : (c + 1) * 128, half:PPAD],
                    )
                # warmup: two back-to-back collectives (absorb start skew and
                # stage the cc queue; their spacing in the trace measures the
                # queue turnaround) + ACT tables (off critical path)
                nc.vector.memset(zsc[:], 0.0)
                nc.sync.dma_start(out=ar0i[:], in_=zsc[:])
                nc.scalar.activation(zsc2[:], zsc[:], AF.Sqrt)
                nc.scalar.activation(zsc2[:], zsc[:], AF.Exp)
                nc.gpsimd.collective_compute(
                    "AllReduce",
                    mybir.AluOpType.add,
                    replica_groups=[list(range(N_CORES))],
                    ins=[ar0i.opt()],
                    outs=[ar0o.opt()],
                )
                nc.gpsimd.collective_compute(
                    "AllReduce",
                    mybir.AluOpType.add,
                    replica_groups=[list(range(N_CORES))],
                    ins=[ar0i.opt()],
                    outs=[ar0o2.opt()],
                )
                # small params + conv2 weights after the conv1-critical loads
                nc.gpsimd.dma_start(out=ident[:], in_=idin.ap()[:])
                nc.gpsimd.dma_start(out=g1b1[:], in_=g1in.ap()[:])
                nc.gpsimd.dma_start(out=g2b2[:], in_=g2in.ap()[:])
                nc.gpsimd.dma_start(out=betav[:], in_=btin.ap()[:])
                nc.gpsimd.dma_start(out=w2sb[:], in_=w2in.ap()[:])

                ps1 = [
                    PS.tile([128, 512], F32, tag="ps", name=f"ps1_{t}")
                    for t in range(NT)
                ]

                def conv1_mm(c, t, o):
                    ky, kx = o // 3, o % 3
                    xv = xsb[c][:].rearrange("p (h w) -> p h w", w=WP)
                    lhsT = w1sb[:, (c * 9 + o) * 128 : (c * 9 + o + 1) * 128]
                    nc.tensor.matmul(
                        ps1[t][:],
                        lhsT,
                        xv[:, 8 * t + ky : 8 * t + ky + 8, kx : kx + W],
                        start=(c == 0 and o == 0),
                        stop=(c == NC1 - 1 and o == 8),
                        skip_group_check=True,
                    )

                # chunks 0..2: weight-stationary order (o outer, t inner)
                for c in range(NC1 - 1):
                    for o in range(9):
                        for t in range(NT):
                            conv1_mm(c, t, o)
                # last chunk: tile-major so tile completions (and stats) spread
                for t in range(NT):
                    for o in range(9):
                        conv1_mm(NC1 - 1, t, o)
                    # drain as soon as this tile's accumulation closes
                    sc = R.tile([128, 512], F32, tag="scr", name=f"scr1_{t}")
                    nc.scalar.activation(
                        out=h1[:, 512 * t : 512 * (t + 1)],
                        in_=ps1[t][:],
                        func=AF.Copy,
                        accum_out=s1[:, t : t + 1],
                    )
                    nc.vector.scalar_tensor_tensor(
                        out=sc[:],
                        in0=ps1[t][:],
                        scalar=1.0,
                        in1=h1[:, 512 * t : 512 * (t + 1)],
                        op0=ALU.mult,
                        op1=ALU.mult,
                        accum_out=q1[:, t : t + 1],
                    )

                # local stats -> AllReduce
                nc.vector.reduce_sum(st1[:, 0:1], s1[:], axis=AX.X)
                nc.vector.reduce_sum(st1[:, 1:2], q1[:], axis=AX.X)
                nc.sync.dma_start(out=ar1i[:], in_=st1[:])
                # preload Sqrt table while the AllReduce flies
                nc.scalar.activation(tbl[:], zsc[:], AF.Sqrt)
                nc.gpsimd.collective_compute(
                    "AllReduce",
                    ALU.add,
                    replica_groups=[list(range(N_CORES))],
                    ins=[ar1i.opt()],
                    outs=[ar1o.opt()],
                )
                nc.gpsimd.dma_start(out=st1g[:], in_=ar1o[:])

                # BN1 coefficients: a = gamma*rsqrt(var+eps), b = beta - mean*a
                mean, ex2, m2, var = co1[:, 0:1], co1[:, 1:2], co1[:, 2:3], co1[:, 3:4]
                sv, rsq, a1 = co1[:, 4:5], co1[:, 5:6], co1[:, 6:7]
                ma, b1 = co1[:, 7:8], co1[:, 8:9]
                nc.scalar.mul(mean, st1g[:, 0:1], inv_n)
                nc.scalar.mul(ex2, st1g[:, 1:2], inv_n)
                nc.scalar.square(m2, mean)
                nc.vector.scalar_tensor_tensor(
                    out=var, in0=ex2, scalar=EPS, in1=m2,
                    op0=ALU.add, op1=ALU.subtract,
                )
                nc.scalar.activation(sv, var, AF.Sqrt)
                nc.vector.reciprocal(rsq, sv)
                nc.vector.tensor_mul(a1, g1b1[:, 0:1], rsq)
                nc.vector.tensor_mul(ma, mean, a1)
                nc.vector.tensor_sub(b1, g1b1[:, 1:2], ma)
                # preload Exp table now (before softmax needs it); scalar is
                # otherwise idle here while vector/gpsimd run the BN applies
                nc.scalar.activation(tbl[:], zsc[:], AF.Exp)

                # BN1 apply + ReLU (mixed engines), then transpose chunks +
                # Gram accumulation on the PE.
                att = PS.tile([128, 512], F32, tag="ps", name="att")
                for t in range(NT):
                    ht = h1[:, 512 * t : 512 * (t + 1)]
                    xt = xf[:, 512 * t : 512 * (t + 1)]
                    if t % 3 == 0:
                        nc.scalar.activation(
                            out=xt, in_=ht, func=AF.Relu, bias=b1, scale=a1,
                        )
                    else:
                        ve = (nc.vector, nc.gpsimd)[t % 3 - 1]
                        ve.tensor_scalar(
                            out=xt, in0=ht,
                            scalar1=a1, scalar2=b1,
                            op0=ALU.mult, op1=ALU.add,
                        )
                        ve.tensor_scalar_max(xt, xt, 0.0)
                    for k in range(4 * t, 4 * t + 4):
                        pst = PS.tile([128, 512], F32, tag="ps",
                                      name=f"pst{k}")
                        pstb = pst[:].bitcast(BF16)[:, 0:128]
                        xfT = P1.tile([128, 128], BF16, tag="xfT", bufs=6,
                                      name=f"xfT{k}")
                        nc.tensor.transpose(
                            pstb, xf[:, 128 * k : 128 * (k + 1)], ident[:]
                        )
                        if k % 2 == 0:
                            nc.vector.tensor_copy(xfT[:], pstb)
                        else:
                            nc.scalar.activation(
                                out=xfT[:], in_=pstb, func=AF.Copy,
                            )
                        nc.tensor.matmul(
                            att[:, 0:128],
                            xfT[:],
                            xfT[:],
                            start=(k == 0),
                            stop=(k == NK - 1),
                            skip_group_check=True,
                        )

                # softmax of (rowmax - att) over rows == exp(rowmin - att)/sum;
                # normalization (and the beta scale) is folded into the output
                # residual op below, so only exp + row-sum happen here.
                amin, asum, arcp, arb = (asml[:, 0:1], asml[:, 1:2],
                                         asml[:, 2:3], asml[:, 3:4])
                nc.vector.tensor_reduce(
                    out=amin, in_=att[:, 0:128], op=ALU.min, axis=AX.X
                )
                nc.scalar.activation(
                    out=att_e[:],
                    in_=att[:, 0:128],
                    func=AF.Exp,
                    bias=amin,
                    scale=-1.0,
                    accum_out=asum,
                )
                nc.vector.reciprocal(arcp, asum)
                nc.vector.tensor_mul(arb, arcp, betav[:])
                pat = PS.tile([128, 512], F32, tag="ps", name="pat")
                patb = pat[:].bitcast(BF16)[:, 0:128]
                nc.tensor.transpose(patb, att_e[:], ident[:])
                nc.vector.tensor_copy(attwT[:], patb)

                # out = (att_e @ xf) * arb + xf  (into padded interior, bf16)
                c2v = c2in[:].rearrange("p (h w) -> p h w", w=WP)
                for t in range(NT):
                    po = PS.tile([128, 512], F32, tag="ps", name=f"po{t}")
                    nc.tensor.matmul(
                        po[:],
                        attwT[:],
                        xf[:, 512 * t : 512 * (t + 1)],
                        start=True, stop=True, skip_group_check=True,
                    )
                    nc.vector.scalar_tensor_tensor(
                        out=c2v[:, 1 + 8 * t : 9 + 8 * t, 1 : 1 + W],
                        in0=po[:],
                        scalar=arb,
                        in1=xf[:, 512 * t : 512 * (t + 1)],
                        op0=ALU.mult,
                        op1=ALU.add,
                    )

            # ---------------- phase 2: conv2, stats AR per chunk ----------------
            with tc.tile_pool(name="phase2", bufs=1) as P2:
                c2vv = c2in[:].rearrange("p (h w) -> p h w", w=WP)
                h2 = [
                    P2.tile([128, PIX], BF16, tag=f"h2_{cc}", name=f"h2_{cc}")
                    for cc in range(NC2)
                ]

                def bn2_coefs(cc):
                    base = 8 * cc
                    mean = co2[:, base + 0 : base + 1]
                    ex2 = co2[:, base + 1 : base + 2]
                    m2 = co2[:, base + 2 : base + 3]
                    var = co2[:, base + 3 : base + 4]
                    sv = co2[:, base + 4 : base + 5]
                    rsq = co2[:, base + 5 : base + 6]
                    a2 = co2[:, base + 6 : base + 7]
                    b2 = co2[:, base + 7 : base + 8]
                    nc.scalar.mul(mean, st2g[:, 2 * cc : 2 * cc + 1], inv_n)
                    nc.scalar.mul(ex2, st2g[:, 2 * cc + 1 : 2 * cc + 2], inv_n)
                    nc.scalar.square(m2, mean)
                    nc.vector.scalar_tensor_tensor(
                        out=var, in0=ex2, scalar=EPS, in1=m2,
                        op0=ALU.add, op1=ALU.subtract,
                    )
                    nc.scalar.activation(sv, var, AF.Sqrt)
                    nc.vector.reciprocal(rsq, sv)
                    nc.vector.tensor_mul(a2, g2b2[:, cc : cc + 1], rsq)
                    nc.vector.tensor_mul(m2, mean, a2)
                    nc.vector.tensor_sub(b2, g2b2[:, 4 + cc : 5 + cc], m2)

                def bn2_apply(cc):
                    a_c = co2[:, 8 * cc + 6 : 8 * cc + 7]
                    b_c = co2[:, 8 * cc + 7 : 8 * cc + 8]
                    for t in range(NT):
                        ob = R.tile([128, 512], F32, tag="ob", bufs=6,
                                    name=f"ob_{cc}_{t}")
                        hsrc = h2[cc][:, 512 * t : 512 * (t + 1)]
                        if t % 3 == 0:
                            nc.scalar.activation(
                                out=ob[:], in_=hsrc, func=AF.Relu,
                                bias=b_c, scale=a_c,
                            )
                        else:
                            ve = (nc.vector, nc.gpsimd)[t % 3 - 1]
                            ve.tensor_scalar(
                                out=ob[:], in0=hsrc,
                                scalar1=a_c, scalar2=b_c,
                                op0=ALU.mult, op1=ALU.add,
                            )
                            ve.tensor_scalar_max(ob[:], ob[:], 0.0)
                        nc.sync.dma_start(
                            out=out.ap()[cc * 128 : (cc + 1) * 128,
                                          512 * t : 512 * (t + 1)],
                            in_=ob[:],
                        )

                def conv2_mm(cc, t, o, ps2):
                    ky, kx = o // 3, o % 3
                    lhsT = w2sb[:, (o * NC2 + cc) * 128
                                : (o * NC2 + cc + 1) * 128]
                    nc.tensor.matmul(
                        ps2[:],
                        lhsT,
                        c2vv[:, 8 * t + ky : 8 * t + ky + 8, kx : kx + W],
                        start=(o == 0),
                        stop=(o == 8),
                        skip_group_check=True,
                    )

                def conv2_drain(cc, t, ps2):
                    sc = R.tile([128, 512], F32, tag="scr",
                                name=f"scr2_{cc}_{t}")
                    nc.scalar.activation(
                        out=h2[cc][:, 512 * t : 512 * (t + 1)],
                        in_=ps2[:],
                        func=AF.Copy,
                        accum_out=s2[:, cc * NT + t : cc * NT + t + 1],
                    )
                    nc.vector.scalar_tensor_tensor(
                        out=sc[:],
                        in0=ps2[:],
                        scalar=1.0,
                        in1=h2[cc][:, 512 * t : 512 * (t + 1)],
                        op0=ALU.mult,
                        op1=ALU.mult,
                        accum_out=q2[:, cc * NT + t : cc * NT + t + 1],
                    )

                def chunk_ar(cc):
                    nc.vector.reduce_sum(
                        st2[:, 2 * cc : 2 * cc + 1],
                        s2[:, cc * NT : (cc + 1) * NT], axis=AX.X,
                    )
                    nc.vector.reduce_sum(
                        st2[:, 2 * cc + 1 : 2 * cc + 2],
                        q2[:, cc * NT : (cc + 1) * NT], axis=AX.X,
                    )
                    nc.sync.dma_start(
                        out=ar2i[cc][:], in_=st2[:, 2 * cc : 2 * cc + 2]
                    )
                    nc.gpsimd.collective_compute(
                        "AllReduce",
                        ALU.add,
                        replica_groups=[list(range(N_CORES))],
                        ins=[ar2i[cc].opt()],
                        outs=[ar2o[cc].opt()],
                    )
                    nc.gpsimd.dma_start(
                        out=st2g[:, 2 * cc : 2 * cc + 2], in_=ar2o[cc][:]
                    )

                for cc in range(NC2):
                    ps2t = [
                        PS.tile([128, 512], F32, tag="ps",
                                name=f"ps2_{cc}_{t}")
                        for t in range(NT)
                    ]
                    if cc < NC2 - 1:
                        # weight-stationary order; drains bunch at chunk end
                        # but overlap the next chunk's matmuls
                        for o in range(9):
                            for t in range(NT):
                                conv2_mm(cc, t, o, ps2t[t])
                        for t in range(NT):
                            conv2_drain(cc, t, ps2t[t])
                    else:
                        # last chunk tile-major: stats ready right at the end
                        for t in range(NT):
                            for o in range(9):
                                conv2_mm(cc, t, o, ps2t[t])
                            conv2_drain(cc, t, ps2t[t])
                    chunk_ar(cc)
                    if cc >= 2:
                        bn2_coefs(cc - 2)
                        bn2_apply(cc - 2)

                for cc in range(NC2 - 2, NC2):
                    bn2_coefs(cc)
                    bn2_apply(cc)
    nc.compile()
    return nc


_CACHE = {}


def _get_nc():
    if "nc" not in _CACHE:
        nc = bacc.Bacc("TRN2", target_bir_lowering=False, debug=False,
                       num_devices=N_CORES)
        _CACHE["nc"] = _emit(nc)
    return _CACHE["nc"]


def _prep_inputs(x, w1, gamma1, bnb1, beta, w2, gamma2, bnb2):
    x = np.asarray(x, np.float32)
    xp = np.zeros((B, CIN, HP, WP), np.float32)
    xp[:, :, 1 : 1 + H, 1 : 1 + W] = x
    xp = xp.reshape(B, CIN, PPAD).astype(BF16_NP)

    w1t = (
        np.asarray(w1, np.float32)
        .transpose(1, 2, 3, 0)                     # [cin, ky, kx, cout]
        .reshape(NC1, 128, 9, 128)                 # [c, p, o, m]
        .transpose(1, 0, 2, 3)                     # [p, c, o, m]
        .reshape(128, NC1 * 9 * 128)
        .astype(BF16_NP)
    )
    w2t = (
        np.asarray(w2, np.float32)
        .transpose(1, 2, 3, 0)                     # [cin=128, ky, kx, cout=512]
        .reshape(128, 9, NC2, 128)                 # [p, o, cc, m]
        .reshape(128, 9 * NC2 * 128)
        .astype(BF16_NP)
    )
    g1b1 = np.stack(
        [np.asarray(gamma1, np.float32), np.asarray(bnb1, np.float32)], axis=1
    )
    g2b2 = np.concatenate(
        [
            np.asarray(gamma2, np.float32).reshape(NC2, 128).T,
            np.asarray(bnb2, np.float32).reshape(NC2, 128).T,
        ],
        axis=1,
    )
    betav = np.full((128, 1), np.asarray(beta, np.float32)[0], np.float32)
    ident = np.eye(128, dtype=BF16_NP)

    shared = {
        "w1t": np.ascontiguousarray(w1t),
        "w2t": np.ascontiguousarray(w2t),
        "g1b1": np.ascontiguousarray(g1b1),
        "g2b2": np.ascontiguousarray(g2b2),
        "betav": betav,
        "ident": ident,
    }
    return [dict(shared, xp=np.ascontiguousarray(xp[b])) for b in range(B)]


def kernel_with_results(inputs, trace=False):
    in_maps = _prep_inputs(**inputs)
    nc = _get_nc()
    res = bass_utils.run_bass_kernel_spmd(
        nc, in_maps, core_ids=list(range(N_CORES)), trace=trace
    )
    outs = np.stack([res.results[b]["out"] for b in range(B)])
    return outs.reshape(B, CIN, H, W).astype(np.float32), res


def kernel(**inputs):
    out, _ = kernel_with_results(inputs, trace=False)
    return out


# revision 11
# speedup vs baseline: 1.1752x; 1.1752x over previous
"""Trainium2 Bass kernel for nn_DAHead_Channel (conv3x3+BN+ReLU -> channel attention -> conv3x3+BN+ReLU).

Data-parallel over batch across 8 NeuronCores (1 batch element each).
Training-mode BatchNorm needs global (N,H,W) stats: conv1 uses one tiny
AllReduce, conv2 fires one AllReduce per output-channel chunk so the collective
latency hides under the next chunk's matmuls.

All matmul operands are bf16 (PSUM accumulation stays fp32). Weight-stationary
loop orders (same lhsT for consecutive matmuls) hit the PE's 216ns/512-row
streaming rate with the LD_WEIGHTS fully hidden; only the last chunk of each
conv runs tile-major so its BN-stats AllReduce can fire immediately.  Collective
warmup (one dummy AllReduce) rides its own DMA queue so the implicit
first-collective barrier starts at t~0 instead of behind the input DMA burst.
GpSimd runs no tensor ops (it is ~10x slower than DVE for them) - only small
DMAs and collective triggers.

Hardcoded problem shape: x [8,512,64,64] f32, w1 [128,512,3,3], w2 [512,128,3,3].
"""

import numpy as np
import ml_dtypes

import concourse.bacc as bacc
import concourse.mybir as mybir
import concourse.tile as tile
from concourse import bass_utils

N_CORES = 8
B, CIN, H, W = 8, 512, 64, 64
HP, WP = H + 2, W + 2          # zero-padded spatial dims
PIX = H * W                    # 4096
PPAD = HP * WP                 # 4356
NT = 8                         # HW tiles of 512 pixels (8 rows of 64)
NC1 = CIN // 128               # 4 input-channel chunks for conv1
NC2 = CIN // 128               # 4 output-channel chunks for conv2
NK = PIX // 128                # 32 transpose chunks for attention
EPS = 1e-5
NHW = B * PIX                  # BN count = 32768
F32 = mybir.dt.float32
BF16 = mybir.dt.bfloat16
BF16_NP = ml_dtypes.bfloat16


def _emit(nc):
    xin = nc.dram_tensor("xp", [CIN, PPAD], BF16, kind="ExternalInput")
    w1in = nc.dram_tensor("w1t", [128, NC1 * 9 * 128], BF16, kind="ExternalInput")
    w2in = nc.dram_tensor("w2t", [128, 9 * NC2 * 128], BF16, kind="ExternalInput")
    g1in = nc.dram_tensor("g1b1", [128, 2], F32, kind="ExternalInput")
    g2in = nc.dram_tensor("g2b2", [128, 8], F32, kind="ExternalInput")
    btin = nc.dram_tensor("betav", [128, 1], F32, kind="ExternalInput")
    idin = nc.dram_tensor("ident", [128, 128], BF16, kind="ExternalInput")
    out = nc.dram_tensor("out", [CIN, PIX], F32, kind="ExternalOutput")

    AF = mybir.ActivationFunctionType
    ALU = mybir.AluOpType
    AX = mybir.AxisListType
    inv_n = 1.0 / NHW

    with tile.TileContext(nc) as tc:
        with (
            tc.tile_pool(name="persist", bufs=1) as P,
            tc.tile_pool(name="rot", bufs=4) as R,
            tc.tile_pool(name="ps", bufs=8, space="PSUM") as PS,
            tc.tile_pool(name="dram", bufs=1, space="DRAM") as DR,
        ):
            # ---------------- persistent SBUF ----------------
            w2sb = P.tile([128, 9 * NC2 * 128], BF16, tag="w2")
            h1 = P.tile([128, PIX], F32, tag="h1")
            xf = P.tile([128, PIX], BF16, tag="xf")
            c2in = P.tile([128, PPAD], BF16, tag="c2in")
            ident = P.tile([128, 128], BF16, tag="ident")
            g1b1 = P.tile([128, 2], F32, tag="g1b1")
            g2b2 = P.tile([128, 8], F32, tag="g2b2")
            betav = P.tile([128, 1], F32, tag="betav")
            s1 = P.tile([128, NT], F32, tag="s1")
            q1 = P.tile([128, NT], F32, tag="q1")
            st1 = P.tile([128, 2], F32, tag="st1")
            st1g = P.tile([128, 2], F32, tag="st1g")
            s2 = P.tile([128, NC2 * NT], F32, tag="s2")
            q2 = P.tile([128, NC2 * NT], F32, tag="q2")
            st2 = P.tile([128, 8], F32, tag="st2")     # per-chunk [sum, sumsq]
            st2g = P.tile([128, 8], F32, tag="st2g")
            co1 = P.tile([128, 10], F32, tag="co1")
            co2 = P.tile([128, 32], F32, tag="co2")    # 8 cols per chunk
            att_e = P.tile([128, 128], BF16, tag="att_e")
            attwT = P.tile([128, 128], BF16, tag="attwT")
            asml = P.tile([128, 4], F32, tag="asml")
            tbl = P.tile([128, 1], F32, tag="tbl")     # act-table warm dummy

            ar0i = DR.tile([128, 1], F32, tag="ar0i")
            ar0o = DR.tile([128, 1], F32, tag="ar0o", addr_space="Shared")
            zsc = P.tile([128, 1], F32, tag="zsc")
            zsc2 = P.tile([128, 1], F32, tag="zsc2")

            ar1i = DR.tile([128, 2], F32, tag="ar1i")
            ar1o = DR.tile([128, 2], F32, tag="ar1o", addr_space="Shared")
            ar2i = [DR.tile([128, 2], F32, tag=f"ar2i{c}", name=f"ar2i{c}")
                    for c in range(NC2)]
            ar2o = [DR.tile([128, 2], F32, tag=f"ar2o{c}", name=f"ar2o{c}",
                            addr_space="Shared")
                    for c in range(NC2)]

            # ---------------- phase 1: conv1 + attention ----------------
            with tc.tile_pool(name="phase1", bufs=1) as P1:
                w1sb = P1.tile([128, NC1 * 9 * 128], BF16, tag="w1")
                xsb = [
                    P1.tile([128, PPAD], BF16, tag=f"x{c}", name=f"xsb{c}")
                    for c in range(NC1)
                ]
                # collective warmup rides the vector DMA queue so the implicit
                # first-collective barrier isn't stuck behind the input burst
                nc.vector.memset(zsc[:], 0.0)
                nc.gpsimd.dma_start(out=ar0i[:], in_=zsc[:])
                nc.gpsimd.collective_compute(
                    "AllReduce",
                    mybir.AluOpType.add,
                    replica_groups=[list(range(N_CORES))],
                    ins=[ar0i.opt()],
                    outs=[ar0o.opt()],
                )
                # conv1-critical loads: w1 chunks on the scalar queue, x on sync
                for c in range(NC1):
                    nc.scalar.dma_start(
                        out=w1sb[:, c * 1152 : (c + 1) * 1152],
                        in_=w1in.ap()[:, c * 1152 : (c + 1) * 1152],
                    )
                    half = PPAD // 2
                    nc.sync.dma_start(
                        out=xsb[c][:, 0:half],
                        in_=xin.ap()[c * 128 : (c + 1) * 128, 0:half],
                    )
                    nc.sync.dma_start(
                        out=xsb[c][:, half:PPAD],
                        in_=xin.ap()[c * 128 : (c + 1) * 128, half:PPAD],
                    )
                # ACT tables + zero the conv2 input borders
                nc.scalar.activation(zsc2[:], zsc[:], AF.Sqrt)
                nc.scalar.activation(zsc2[:], zsc[:], AF.Exp)
                nc.vector.memset(c2in[:], 0.0)
                # small params + conv2 weights (gpsimd queue, after trigger)
                nc.gpsimd.dma_start(out=ident[:], in_=idin.ap()[:])
                nc.gpsimd.dma_start(out=g1b1[:], in_=g1in.ap()[:])
                nc.gpsimd.dma_start(out=g2b2[:], in_=g2in.ap()[:])
                nc.gpsimd.dma_start(out=betav[:], in_=btin.ap()[:])
                nc.gpsimd.dma_start(out=w2sb[:], in_=w2in.ap()[:])

                ps1 = [
                    PS.tile([128, 512], F32, tag="ps", name=f"ps1_{t}")
                    for t in range(NT)
                ]

                def conv1_mm(c, t, o):
                    ky, kx = o // 3, o % 3
                    xv = xsb[c][:].rearrange("p (h w) -> p h w", w=WP)
                    lhsT = w1sb[:, (c * 9 + o) * 128 : (c * 9 + o + 1) * 128]
                    nc.tensor.matmul(
                        ps1[t][:],
                        lhsT,
                        xv[:, 8 * t + ky : 8 * t + ky + 8, kx : kx + W],
                        start=(c == 0 and o == 0),
                        stop=(c == NC1 - 1 and o == 8),
                        skip_group_check=True,
                    )

                # chunks 0..2: weight-stationary order (o outer, t inner)
                for c in range(NC1 - 1):
                    for o in range(9):
                        for t in range(NT):
                            conv1_mm(c, t, o)
                # last chunk: tile-major so tile completions (and stats) spread
                for t in range(NT):
                    for o in range(9):
                        conv1_mm(NC1 - 1, t, o)
                    # drain as soon as this tile's accumulation closes
                    sc = R.tile([128, 512], F32, tag="scr", name=f"scr1_{t}")
                    nc.scalar.activation(
                        out=h1[:, 512 * t : 512 * (t + 1)],
                        in_=ps1[t][:],
                        func=AF.Copy,
                        accum_out=s1[:, t : t + 1],
                    )
                    nc.vector.scalar_tensor_tensor(
                        out=sc[:],
                        in0=ps1[t][:],
                        scalar=1.0,
                        in1=h1[:, 512 * t : 512 * (t + 1)],
                        op0=ALU.mult,
                        op1=ALU.mult,
                        accum_out=q1[:, t : t + 1],
                    )

                # local stats -> AllReduce
                nc.vector.reduce_sum(st1[:, 0:1], s1[:], axis=AX.X)
                nc.vector.reduce_sum(st1[:, 1:2], q1[:], axis=AX.X)
                nc.sync.dma_start(out=ar1i[:], in_=st1[:])
                nc.gpsimd.collective_compute(
                    "AllReduce",
                    ALU.add,
                    replica_groups=[list(range(N_CORES))],
                    ins=[ar1i.opt()],
                    outs=[ar1o.opt()],
                )
                nc.gpsimd.dma_start(out=st1g[:], in_=ar1o[:])

                # BN1 coefficients: a = gamma*rsqrt(var+eps), b = beta - mean*a
                mean, ex2, m2, var = co1[:, 0:1], co1[:, 1:2], co1[:, 2:3], co1[:, 3:4]
                sv, rsq, a1 = co1[:, 4:5], co1[:, 5:6], co1[:, 6:7]
                ma, b1 = co1[:, 7:8], co1[:, 8:9]
                nc.scalar.mul(mean, st1g[:, 0:1], inv_n)
                nc.scalar.mul(ex2, st1g[:, 1:2], inv_n)
                nc.scalar.square(m2, mean)
                nc.vector.scalar_tensor_tensor(
                    out=var, in0=ex2, scalar=EPS, in1=m2,
                    op0=ALU.add, op1=ALU.subtract,
                )
                nc.scalar.activation(sv, var, AF.Sqrt)
                nc.vector.reciprocal(rsq, sv)
                nc.vector.tensor_mul(a1, g1b1[:, 0:1], rsq)
                nc.vector.tensor_mul(ma, mean, a1)
                nc.vector.tensor_sub(b1, g1b1[:, 1:2], ma)

                # BN1 apply + ReLU (scalar/vector alternating), then transpose
                # chunks + Gram accumulation on the PE.
                att = PS.tile([128, 512], F32, tag="ps", name="att")
                for t in range(NT):
                    ht = h1[:, 512 * t : 512 * (t + 1)]
                    xt = xf[:, 512 * t : 512 * (t + 1)]
                    if t % 2 == 0:
                        nc.scalar.activation(
                            out=xt, in_=ht, func=AF.Relu, bias=b1, scale=a1,
                        )
                    else:
                        nc.vector.tensor_scalar(
                            out=xt, in0=ht,
                            scalar1=a1, scalar2=b1,
                            op0=ALU.mult, op1=ALU.add,
                        )
                        nc.vector.tensor_scalar_max(xt, xt, 0.0)
                    if t == NT - 1:
                        # preload the Exp table for softmax while the PE works
                        nc.scalar.activation(tbl[:], zsc[:], AF.Exp)
                    for k in range(4 * t, 4 * t + 4):
                        pst = PS.tile([128, 512], F32, tag="ps",
                                      name=f"pst{k}")
                        pstb = pst[:].bitcast(BF16)[:, 0:128]
                        xfT = P1.tile([128, 128], BF16, tag="xfT", bufs=6,
                                      name=f"xfT{k}")
                        nc.tensor.transpose(
                            pstb, xf[:, 128 * k : 128 * (k + 1)], ident[:]
                        )
                        if k % 2 == 0:
                            nc.vector.tensor_copy(xfT[:], pstb)
                        else:
                            nc.scalar.activation(
                                out=xfT[:], in_=pstb, func=AF.Copy,
                            )
                        nc.tensor.matmul(
                            att[:, 0:128],
                            xfT[:],
                            xfT[:],
                            start=(k == 0),
                            stop=(k == NK - 1),
                            skip_group_check=True,
                        )

                # softmax of (rowmax - att) over rows == exp(rowmin - att)/sum;
                # normalization (and the beta scale) is folded into the output
                # residual op below, so only exp + row-sum happen here.
                amin, asum, arcp, arb = (asml[:, 0:1], asml[:, 1:2],
                                         asml[:, 2:3], asml[:, 3:4])
                nc.vector.tensor_reduce(
                    out=amin, in_=att[:, 0:128], op=ALU.min, axis=AX.X
                )
                nc.scalar.activation(
                    out=att_e[:],
                    in_=att[:, 0:128],
                    func=AF.Exp,
                    bias=amin,
                    scale=-1.0,
                    accum_out=asum,
                )
                nc.vector.reciprocal(arcp, asum)
                nc.vector.tensor_mul(arb, arcp, betav[:])
                pat = PS.tile([128, 512], F32, tag="ps", name="pat")
                patb = pat[:].bitcast(BF16)[:, 0:128]
                nc.tensor.transpose(patb, att_e[:], ident[:])
                nc.vector.tensor_copy(attwT[:], patb)

                # out = (att_e @ xf) * arb + xf  (into padded interior, bf16)
                c2v = c2in[:].rearrange("p (h w) -> p h w", w=WP)
                for t in range(NT):
                    po = PS.tile([128, 512], F32, tag="ps", name=f"po{t}")
                    nc.tensor.matmul(
                        po[:],
                        attwT[:],
                        xf[:, 512 * t : 512 * (t + 1)],
                        start=True, stop=True, skip_group_check=True,
                    )
                    nc.vector.scalar_tensor_tensor(
                        out=c2v[:, 1 + 8 * t : 9 + 8 * t, 1 : 1 + W],
                        in0=po[:],
                        scalar=arb,
                        in1=xf[:, 512 * t : 512 * (t + 1)],
                        op0=ALU.mult,
                        op1=ALU.add,
                    )

            # ---------------- phase 2: conv2, stats AR per chunk ----------------
            with tc.tile_pool(name="phase2", bufs=1) as P2:
                c2vv = c2in[:].rearrange("p (h w) -> p h w", w=WP)
                h2 = [
                    P2.tile([128, PIX], BF16, tag=f"h2_{cc}", name=f"h2_{cc}")
                    for cc in range(NC2)
                ]

                def bn2_coefs(cc):
                    base = 8 * cc
                    mean = co2[:, base + 0 : base + 1]
                    ex2 = co2[:, base + 1 : base + 2]
                    m2 = co2[:, base + 2 : base + 3]
                    var = co2[:, base + 3 : base + 4]
                    sv = co2[:, base + 4 : base + 5]
                    rsq = co2[:, base + 5 : base + 6]
                    a2 = co2[:, base + 6 : base + 7]
                    b2 = co2[:, base + 7 : base + 8]
                    nc.scalar.mul(mean, st2g[:, 2 * cc : 2 * cc + 1], inv_n)
                    nc.scalar.mul(ex2, st2g[:, 2 * cc + 1 : 2 * cc + 2], inv_n)
                    nc.scalar.square(m2, mean)
                    nc.vector.scalar_tensor_tensor(
                        out=var, in0=ex2, scalar=EPS, in1=m2,
                        op0=ALU.add, op1=ALU.subtract,
                    )
                    nc.scalar.activation(sv, var, AF.Sqrt)
                    nc.vector.reciprocal(rsq, sv)
                    nc.vector.tensor_mul(a2, g2b2[:, cc : cc + 1], rsq)
                    nc.vector.tensor_mul(m2, mean, a2)
                    nc.vector.tensor_sub(b2, g2b2[:, 4 + cc : 5 + cc], m2)

                def bn2_apply(cc):
                    a_c = co2[:, 8 * cc + 6 : 8 * cc + 7]
                    b_c = co2[:, 8 * cc + 7 : 8 * cc + 8]
                    for t in range(NT):
                        ob = R.tile([128, 512], F32, tag="ob", bufs=6,
                                    name=f"ob_{cc}_{t}")
                        hsrc = h2[cc][:, 512 * t : 512 * (t + 1)]
                        if t % 2 == 0:
                            nc.scalar.activation(
                                out=ob[:], in_=hsrc, func=AF.Relu,
                                bias=b_c, scale=a_c,
                            )
                        else:
                            nc.vector.tensor_scalar(
                                out=ob[:], in0=hsrc,
                                scalar1=a_c, scalar2=b_c,
                                op0=ALU.mult, op1=ALU.add,
                            )
                            nc.vector.tensor_scalar_max(ob[:], ob[:], 0.0)
                        nc.sync.dma_start(
                            out=out.ap()[cc * 128 : (cc + 1) * 128,
                                          512 * t : 512 * (t + 1)],
                            in_=ob[:],
                        )

                def conv2_mm(cc, t, o, ps2):
                    ky, kx = o // 3, o % 3
                    lhsT = w2sb[:, (o * NC2 + cc) * 128
                                : (o * NC2 + cc + 1) * 128]
                    nc.tensor.matmul(
                        ps2[:],
                        lhsT,
                        c2vv[:, 8 * t + ky : 8 * t + ky + 8, kx : kx + W],
                        start=(o == 0),
                        stop=(o == 8),
                        skip_group_check=True,
                    )

                def conv2_drain(cc, t, ps2):
                    sc = R.tile([128, 512], F32, tag="scr",
                                name=f"scr2_{cc}_{t}")
                    nc.scalar.activation(
                        out=h2[cc][:, 512 * t : 512 * (t + 1)],
                        in_=ps2[:],
                        func=AF.Copy,
                        accum_out=s2[:, cc * NT + t : cc * NT + t + 1],
                    )
                    nc.vector.scalar_tensor_tensor(
                        out=sc[:],
                        in0=ps2[:],
                        scalar=1.0,
                        in1=h2[cc][:, 512 * t : 512 * (t + 1)],
                        op0=ALU.mult,
                        op1=ALU.mult,
                        accum_out=q2[:, cc * NT + t : cc * NT + t + 1],
                    )

                def chunk_ar(cc):
                    nc.vector.reduce_sum(
                        st2[:, 2 * cc : 2 * cc + 1],
                        s2[:, cc * NT : (cc + 1) * NT], axis=AX.X,
                    )
                    nc.vector.reduce_sum(
                        st2[:, 2 * cc + 1 : 2 * cc + 2],
                        q2[:, cc * NT : (cc + 1) * NT], axis=AX.X,
                    )
                    nc.sync.dma_start(
                        out=ar2i[cc][:], in_=st2[:, 2 * cc : 2 * cc + 2]
                    )
                    nc.gpsimd.collective_compute(
                        "AllReduce",
                        ALU.add,
                        replica_groups=[list(range(N_CORES))],
                        ins=[ar2i[cc].opt()],
                        outs=[ar2o[cc].opt()],
                    )
                    nc.gpsimd.dma_start(
                        out=st2g[:, 2 * cc : 2 * cc + 2], in_=ar2o[cc][:]
                    )

                for cc in range(NC2):
                    ps2t = [
                        PS.tile([128, 512], F32, tag="ps",
                                name=f"ps2_{cc}_{t}")
                        for t in range(NT)
                    ]
                    if cc < NC2 - 1:
                        # weight-stationary in two half-sets of 4 PSUM banks so
                        # the next chunk's first pass never waits on drains
                        for hf in range(2):
                            for o in range(9):
                                for t in range(4 * hf, 4 * hf + 4):
                                    conv2_mm(cc, t, o, ps2t[t])
                            for t in range(4 * hf, 4 * hf + 4):
                                conv2_drain(cc, t, ps2t[t])
                    else:
                        # last chunk tile-major: stats ready right at the end
                        for t in range(NT):
                            for o in range(9):
                                conv2_mm(cc, t, o, ps2t[t])
                            conv2_drain(cc, t, ps2t[t])
                    chunk_ar(cc)
                    if cc >= 2:
                        bn2_coefs(cc - 2)
                        bn2_apply(cc - 2)

                for cc in range(NC2 - 2, NC2):
                    bn2_coefs(cc)
                    bn2_apply(cc)
    nc.compile()
    return nc


_CACHE = {}


def _get_nc():
    if "nc" not in _CACHE:
        nc = bacc.Bacc("TRN2", target_bir_lowering=False, debug=False,
                       num_devices=N_CORES)
        _CACHE["nc"] = _emit(nc)
    return _CACHE["nc"]


def _prep_inputs(x, w1, gamma1, bnb1, beta, w2, gamma2, bnb2):
    x = np.asarray(x, np.float32)
    xp = np.zeros((B, CIN, HP, WP), np.float32)
    xp[:, :, 1 : 1 + H, 1 : 1 + W] = x
    xp = xp.reshape(B, CIN, PPAD).astype(BF16_NP)

    w1t = (
        np.asarray(w1, np.float32)
        .transpose(1, 2, 3, 0)                     # [cin, ky, kx, cout]
        .reshape(NC1, 128, 9, 128)                 # [c, p, o, m]
        .transpose(1, 0, 2, 3)                     # [p, c, o, m]
        .reshape(128, NC1 * 9 * 128)
        .astype(BF16_NP)
    )
    w2t = (
        np.asarray(w2, np.float32)
        .transpose(1, 2, 3, 0)                     # [cin=128, ky, kx, cout=512]
        .reshape(128, 9, NC2, 128)                 # [p, o, cc, m]
        .reshape(128, 9 * NC2 * 128)
        .astype(BF16_NP)
    )
    g1b1 = np.stack(
        [np.asarray(gamma1, np.float32), np.asarray(bnb1, np.float32)], axis=1
    )
    g2b2 = np.concatenate(
        [
            np.asarray(gamma2, np.float32).reshape(NC2, 128).T,
            np.asarray(bnb2, np.float32).reshape(NC2, 128).T,
        ],
        axis=1,
    )
    betav = np.full((128, 1), np.asarray(beta, np.float32)[0], np.float32)
    ident = np.eye(128, dtype=BF16_NP)

    shared = {
        "w1t": np.ascontiguousarray(w1t),
        "w2t": np.ascontiguousarray(w2t),
        "g1b1": np.ascontiguousarray(g1b1),
        "g2b2": np.ascontiguousarray(g2b2),
        "betav": betav,
        "ident": ident,
    }
    return [dict(shared, xp=np.ascontiguousarray(xp[b])) for b in range(B)]


def kernel_with_results(inputs, trace=False):
    in_maps = _prep_inputs(**inputs)
    nc = _get_nc()
    res = bass_utils.run_bass_kernel_spmd(
        nc, in_maps, core_ids=list(range(N_CORES)), trace=trace
    )
    outs = np.stack([res.results[b]["out"] for b in range(B)])
    return outs.reshape(B, CIN, H, W).astype(np.float32), res


def kernel(**inputs):
    out, _ = kernel_with_results(inputs, trace=False)
    return out


# revision 18
# speedup vs baseline: 1.1818x; 1.0056x over previous
"""Trainium2 Bass kernel for nn_DAHead_Channel (conv3x3+BN+ReLU -> channel attention -> conv3x3+BN+ReLU).

Data-parallel over batch across 8 NeuronCores (1 batch element each).
Training-mode BatchNorm needs global (N,H,W) stats: conv1 uses one tiny
AllReduce, conv2 fires one AllReduce per output-channel chunk so the collective
latency hides under the next chunk's matmuls.

All matmul operands are bf16 (PSUM accumulation stays fp32). Weight-stationary
loop orders (same lhsT for consecutive matmuls) hit the PE's 216ns/512-row
streaming rate with the LD_WEIGHTS fully hidden; only the last chunk of each
conv runs tile-major so its BN-stats AllReduce can fire immediately.  Collective
warmup (one dummy AllReduce) rides its own DMA queue so the implicit
first-collective barrier starts at t~0 instead of behind the input DMA burst.
GpSimd runs no tensor ops (it is ~10x slower than DVE for them) - only small
DMAs and collective triggers.

Hardcoded problem shape: x [8,512,64,64] f32, w1 [128,512,3,3], w2 [512,128,3,3].
"""

import numpy as np
import ml_dtypes

import concourse.bacc as bacc
import concourse.mybir as mybir
import concourse.tile as tile
from concourse import bass_utils

N_CORES = 8
B, CIN, H, W = 8, 512, 64, 64
HP, WP = H + 2, W + 2          # zero-padded spatial dims
PIX = H * W                    # 4096
PPAD = HP * WP                 # 4356
NT = 8                         # HW tiles of 512 pixels (8 rows of 64)
NC1 = CIN // 128               # 4 input-channel chunks for conv1
NC2 = CIN // 128               # 4 output-channel chunks for conv2
NK = PIX // 128                # 32 transpose chunks for attention
EPS = 1e-5
NHW = B * PIX                  # BN count = 32768
F32 = mybir.dt.float32
BF16 = mybir.dt.bfloat16
BF16_NP = ml_dtypes.bfloat16


def _emit(nc):
    xin = nc.dram_tensor("xp", [CIN, PPAD], BF16, kind="ExternalInput")
    w1in = nc.dram_tensor("w1t", [128, NC1 * 9 * 128], BF16, kind="ExternalInput")
    w2in = nc.dram_tensor("w2t", [128, 9 * NC2 * 128], BF16, kind="ExternalInput")
    g1in = nc.dram_tensor("g1b1", [128, 2], F32, kind="ExternalInput")
    g2in = nc.dram_tensor("g2b2", [128, 8], F32, kind="ExternalInput")
    btin = nc.dram_tensor("betav", [128, 1], F32, kind="ExternalInput")
    idin = nc.dram_tensor("ident", [128, 128], BF16, kind="ExternalInput")
    out = nc.dram_tensor("out", [CIN, PIX], F32, kind="ExternalOutput")

    AF = mybir.ActivationFunctionType
    ALU = mybir.AluOpType
    AX = mybir.AxisListType
    inv_n = 1.0 / NHW

    with tile.TileContext(nc) as tc:
        with (
            tc.tile_pool(name="persist", bufs=1) as P,
            tc.tile_pool(name="rot", bufs=4) as R,
            tc.tile_pool(name="ps", bufs=8, space="PSUM") as PS,
            tc.tile_pool(name="dram", bufs=1, space="DRAM") as DR,
        ):
            # ---------------- persistent SBUF ----------------
            w2sb = P.tile([128, 9 * NC2 * 128], BF16, tag="w2")
            h1 = P.tile([128, PIX], F32, tag="h1")
            xf = P.tile([128, PIX], BF16, tag="xf")
            c2in = P.tile([128, PPAD], BF16, tag="c2in")
            ident = P.tile([128, 128], BF16, tag="ident")
            g1b1 = P.tile([128, 2], F32, tag="g1b1")
            g2b2 = P.tile([128, 8], F32, tag="g2b2")
            betav = P.tile([128, 1], F32, tag="betav")
            s1 = P.tile([128, NT], F32, tag="s1")
            q1 = P.tile([128, NT], F32, tag="q1")
            st1 = P.tile([128, 2], F32, tag="st1")
            st1g = P.tile([128, 2], F32, tag="st1g")
            s2 = P.tile([128, NC2 * NT], F32, tag="s2")
            q2 = P.tile([128, NC2 * NT], F32, tag="q2")
            st2 = P.tile([128, 8], F32, tag="st2")     # per-chunk [sum, sumsq]
            st2g = P.tile([128, 8], F32, tag="st2g")
            co1 = P.tile([128, 10], F32, tag="co1")
            co2 = P.tile([128, 32], F32, tag="co2")    # 8 cols per chunk
            att_e = P.tile([128, 128], BF16, tag="att_e")
            attwT = P.tile([128, 128], BF16, tag="attwT")
            asml = P.tile([128, 4], F32, tag="asml")
            tbl = P.tile([128, 1], F32, tag="tbl")     # act-table warm dummy

            ar0i = DR.tile([128, 1], F32, tag="ar0i")
            ar0o = DR.tile([128, 1], F32, tag="ar0o", addr_space="Shared")
            zsc = P.tile([128, 1], F32, tag="zsc")
            zsc2 = P.tile([128, 1], F32, tag="zsc2")

            ar1i = DR.tile([128, 2], F32, tag="ar1i")
            ar1o = DR.tile([128, 2], F32, tag="ar1o", addr_space="Shared")
            ar2i = [DR.tile([128, 2], F32, tag=f"ar2i{c}", name=f"ar2i{c}")
                    for c in range(NC2)]
            ar2o = [DR.tile([128, 2], F32, tag=f"ar2o{c}", name=f"ar2o{c}",
                            addr_space="Shared")
                    for c in range(NC2)]

            # ---------------- phase 1: conv1 + attention ----------------
            with tc.tile_pool(name="phase1", bufs=1) as P1:
                w1sb = P1.tile([128, NC1 * 9 * 128], BF16, tag="w1")
                xsb = [
                    P1.tile([128, PPAD], BF16, tag=f"x{c}", name=f"xsb{c}")
                    for c in range(NC1)
                ]
                # collective warmup rides the vector DMA queue so the implicit
                # first-collective barrier isn't stuck behind the input burst
                nc.vector.memset(zsc[:], 0.0)
                nc.gpsimd.dma_start(out=ar0i[:], in_=zsc[:])
                nc.gpsimd.collective_compute(
                    "AllReduce",
                    mybir.AluOpType.add,
                    replica_groups=[list(range(N_CORES))],
                    ins=[ar0i.opt()],
                    outs=[ar0o.opt()],
                )
                # conv1-critical loads, all on the sync queue in consumption
                # order: w1 chunk 0 first, then x chunk 0 in quarters (chunk 0
                # runs tile-major so the PE chases the DMA down the rows)
                nc.sync.dma_start(
                    out=w1sb[:, 0:1152], in_=w1in.ap()[:, 0:1152],
                )
                quart = PPAD // 4
                for qq in range(4):
                    lo, hi = qq * quart, (qq + 1) * quart if qq < 3 else PPAD
                    nc.sync.dma_start(
                        out=xsb[0][:, lo:hi], in_=xin.ap()[0:128, lo:hi],
                    )
                for c in range(1, NC1):
                    nc.sync.dma_start(
                        out=w1sb[:, c * 1152 : (c + 1) * 1152],
                        in_=w1in.ap()[:, c * 1152 : (c + 1) * 1152],
                    )
                    half = PPAD // 2
                    nc.sync.dma_start(
                        out=xsb[c][:, 0:half],
                        in_=xin.ap()[c * 128 : (c + 1) * 128, 0:half],
                    )
                    nc.sync.dma_start(
                        out=xsb[c][:, half:PPAD],
                        in_=xin.ap()[c * 128 : (c + 1) * 128, half:PPAD],
                    )
                # ACT tables + zero the conv2 input borders
                nc.scalar.activation(zsc2[:], zsc[:], AF.Sqrt)
                nc.scalar.activation(zsc2[:], zsc[:], AF.Exp)
                nc.vector.memset(c2in[:], 0.0)
                # small params + conv2 weights (gpsimd queue, after trigger)
                nc.gpsimd.dma_start(out=ident[:], in_=idin.ap()[:])
                nc.gpsimd.dma_start(out=g1b1[:], in_=g1in.ap()[:])
                nc.gpsimd.dma_start(out=g2b2[:], in_=g2in.ap()[:])
                nc.gpsimd.dma_start(out=betav[:], in_=btin.ap()[:])
                nc.gpsimd.dma_start(out=w2sb[:], in_=w2in.ap()[:])

                ps1 = [
                    PS.tile([128, 512], F32, tag="ps", name=f"ps1_{t}")
                    for t in range(NT)
                ]

                def conv1_mm(c, t, o):
                    ky, kx = o // 3, o % 3
                    xv = xsb[c][:].rearrange("p (h w) -> p h w", w=WP)
                    lhsT = w1sb[:, (c * 9 + o) * 128 : (c * 9 + o + 1) * 128]
                    nc.tensor.matmul(
                        ps1[t][:],
                        lhsT,
                        xv[:, 8 * t + ky : 8 * t + ky + 8, kx : kx + W],
                        start=(c == 0 and o == 0),
                        stop=(c == NC1 - 1 and o == 8),
                        skip_group_check=True,
                    )

                # chunk 0 tile-major (follows the DMA down the rows); chunks
                # 1..2 weight-stationary (o outer, t inner)
                for t in range(NT):
                    for o in range(9):
                        conv1_mm(0, t, o)
                for c in range(1, NC1 - 1):
                    for o in range(9):
                        for t in range(NT):
                            conv1_mm(c, t, o)
                # last chunk: tile-major so tile completions (and stats) spread
                for t in range(NT):
                    for o in range(9):
                        conv1_mm(NC1 - 1, t, o)
                    # drain as soon as this tile's accumulation closes
                    sc = R.tile([128, 512], F32, tag="scr", name=f"scr1_{t}")
                    nc.scalar.activation(
                        out=h1[:, 512 * t : 512 * (t + 1)],
                        in_=ps1[t][:],
                        func=AF.Copy,
                        accum_out=s1[:, t : t + 1],
                    )
                    nc.vector.scalar_tensor_tensor(
                        out=sc[:],
                        in0=ps1[t][:],
                        scalar=1.0,
                        in1=h1[:, 512 * t : 512 * (t + 1)],
                        op0=ALU.mult,
                        op1=ALU.mult,
                        accum_out=q1[:, t : t + 1],
                    )

                # local stats -> AllReduce (stats DMA on the scalar queue:
                # never behind bulk transfers)
                nc.vector.reduce_sum(st1[:, 0:1], s1[:], axis=AX.X)
                nc.vector.reduce_sum(st1[:, 1:2], q1[:], axis=AX.X)
                nc.scalar.dma_start(out=ar1i[:], in_=st1[:])
                nc.gpsimd.collective_compute(
                    "AllReduce",
                    ALU.add,
                    replica_groups=[list(range(N_CORES))],
                    ins=[ar1i.opt()],
                    outs=[ar1o.opt()],
                )
                nc.gpsimd.dma_start(out=st1g[:], in_=ar1o[:])

                # BN1 coefficients: a = gamma*rsqrt(var+eps), b = beta - mean*a
                mean, ex2, m2, var = co1[:, 0:1], co1[:, 1:2], co1[:, 2:3], co1[:, 3:4]
                sv, rsq, a1 = co1[:, 4:5], co1[:, 5:6], co1[:, 6:7]
                ma, b1 = co1[:, 7:8], co1[:, 8:9]
                nc.scalar.mul(mean, st1g[:, 0:1], inv_n)
                nc.scalar.mul(ex2, st1g[:, 1:2], inv_n)
                nc.scalar.square(m2, mean)
                nc.vector.scalar_tensor_tensor(
                    out=var, in0=ex2, scalar=EPS, in1=m2,
                    op0=ALU.add, op1=ALU.subtract,
                )
                nc.scalar.activation(sv, var, AF.Sqrt)
                nc.vector.reciprocal(rsq, sv)
                nc.vector.tensor_mul(a1, g1b1[:, 0:1], rsq)
                nc.vector.tensor_mul(ma, mean, a1)
                nc.vector.tensor_sub(b1, g1b1[:, 1:2], ma)

                # BN1 apply + ReLU (scalar/vector alternating), then transpose
                # chunks + Gram accumulation on the PE.
                att = PS.tile([128, 512], F32, tag="ps", name="att")
                for t in range(NT):
                    ht = h1[:, 512 * t : 512 * (t + 1)]
                    xt = xf[:, 512 * t : 512 * (t + 1)]
                    if t % 2 == 0:
                        nc.scalar.activation(
                            out=xt, in_=ht, func=AF.Relu, bias=b1, scale=a1,
                        )
                    else:
                        nc.vector.tensor_scalar(
                            out=xt, in0=ht,
                            scalar1=a1, scalar2=b1,
                            op0=ALU.mult, op1=ALU.add,
                        )
                        nc.vector.tensor_scalar_max(xt, xt, 0.0)
                    if t == NT - 1:
                        # preload the Exp table for softmax while the PE works
                        nc.scalar.activation(tbl[:], zsc[:], AF.Exp)
                    for k in range(4 * t, 4 * t + 4):
                        pst = PS.tile([128, 512], F32, tag="ps",
                                      name=f"pst{k}")
                        pstb = pst[:].bitcast(BF16)[:, 0:128]
                        xfT = P1.tile([128, 128], BF16, tag="xfT", bufs=6,
                                      name=f"xfT{k}")
                        nc.tensor.transpose(
                            pstb, xf[:, 128 * k : 128 * (k + 1)], ident[:]
                        )
                        if k % 2 == 0:
                            nc.vector.tensor_copy(xfT[:], pstb)
                        else:
                            nc.scalar.activation(
                                out=xfT[:], in_=pstb, func=AF.Copy,
                            )
                        nc.tensor.matmul(
                            att[:, 0:128],
                            xfT[:],
                            xfT[:],
                            start=(k == 0),
                            stop=(k == NK - 1),
                            skip_group_check=True,
                        )

                # softmax of (rowmax - att) over rows == exp(rowmin - att)/sum;
                # normalization (and the beta scale) is folded into the output
                # residual op below, so only exp + row-sum happen here.
                amin, asum, arcp, arb = (asml[:, 0:1], asml[:, 1:2],
                                         asml[:, 2:3], asml[:, 3:4])
                nc.vector.tensor_reduce(
                    out=amin, in_=att[:, 0:128], op=ALU.min, axis=AX.X
                )
                nc.scalar.activation(
                    out=att_e[:],
                    in_=att[:, 0:128],
                    func=AF.Exp,
                    bias=amin,
                    scale=-1.0,
                    accum_out=asum,
                )
                nc.vector.reciprocal(arcp, asum)
                nc.vector.tensor_mul(arb, arcp, betav[:])
                pat = PS.tile([128, 512], F32, tag="ps", name="pat")
                patb = pat[:].bitcast(BF16)[:, 0:128]
                nc.tensor.transpose(patb, att_e[:], ident[:])
                nc.vector.tensor_copy(attwT[:], patb)

                # out = (att_e @ xf) * arb + xf  (into padded interior, bf16)
                c2v = c2in[:].rearrange("p (h w) -> p h w", w=WP)
                for t in range(NT):
                    po = PS.tile([128, 512], F32, tag="ps", name=f"po{t}")
                    nc.tensor.matmul(
                        po[:],
                        attwT[:],
                        xf[:, 512 * t : 512 * (t + 1)],
                        start=True, stop=True, skip_group_check=True,
                    )
                    nc.vector.scalar_tensor_tensor(
                        out=c2v[:, 1 + 8 * t : 9 + 8 * t, 1 : 1 + W],
                        in0=po[:],
                        scalar=arb,
                        in1=xf[:, 512 * t : 512 * (t + 1)],
                        op0=ALU.mult,
                        op1=ALU.add,
                    )

            # ---------------- phase 2: conv2, stats AR per chunk ----------------
            with tc.tile_pool(name="phase2", bufs=1) as P2:
                c2vv = c2in[:].rearrange("p (h w) -> p h w", w=WP)
                h2 = [
                    P2.tile([128, PIX], BF16, tag=f"h2_{cc}", name=f"h2_{cc}")
                    for cc in range(NC2)
                ]

                def bn2_coefs(cc):
                    base = 8 * cc
                    mean = co2[:, base + 0 : base + 1]
                    ex2 = co2[:, base + 1 : base + 2]
                    m2 = co2[:, base + 2 : base + 3]
                    var = co2[:, base + 3 : base + 4]
                    sv = co2[:, base + 4 : base + 5]
                    rsq = co2[:, base + 5 : base + 6]
                    a2 = co2[:, base + 6 : base + 7]
                    b2 = co2[:, base + 7 : base + 8]
                    nc.scalar.mul(mean, st2g[:, 2 * cc : 2 * cc + 1], inv_n)
                    nc.scalar.mul(ex2, st2g[:, 2 * cc + 1 : 2 * cc + 2], inv_n)
                    nc.scalar.square(m2, mean)
                    nc.vector.scalar_tensor_tensor(
                        out=var, in0=ex2, scalar=EPS, in1=m2,
                        op0=ALU.add, op1=ALU.subtract,
                    )
                    nc.scalar.activation(sv, var, AF.Sqrt)
                    nc.vector.reciprocal(rsq, sv)
                    nc.vector.tensor_mul(a2, g2b2[:, cc : cc + 1], rsq)
                    nc.vector.tensor_mul(m2, mean, a2)
                    nc.vector.tensor_sub(b2, g2b2[:, 4 + cc : 5 + cc], m2)

                def bn2_apply(cc):
                    a_c = co2[:, 8 * cc + 6 : 8 * cc + 7]
                    b_c = co2[:, 8 * cc + 7 : 8 * cc + 8]
                    for t in range(NT):
                        ob = R.tile([128, 512], F32, tag="ob", bufs=6,
                                    name=f"ob_{cc}_{t}")
                        hsrc = h2[cc][:, 512 * t : 512 * (t + 1)]
                        if t % 2 == 0:
                            nc.scalar.activation(
                                out=ob[:], in_=hsrc, func=AF.Relu,
                                bias=b_c, scale=a_c,
                            )
                        else:
                            nc.vector.tensor_scalar(
                                out=ob[:], in0=hsrc,
                                scalar1=a_c, scalar2=b_c,
                                op0=ALU.mult, op1=ALU.add,
                            )
                            nc.vector.tensor_scalar_max(ob[:], ob[:], 0.0)
                        nc.sync.dma_start(
                            out=out.ap()[cc * 128 : (cc + 1) * 128,
                                          512 * t : 512 * (t + 1)],
                            in_=ob[:],
                        )

                def conv2_mm(cc, t, o, ps2):
                    ky, kx = o // 3, o % 3
                    lhsT = w2sb[:, (o * NC2 + cc) * 128
                                : (o * NC2 + cc + 1) * 128]
                    nc.tensor.matmul(
                        ps2[:],
                        lhsT,
                        c2vv[:, 8 * t + ky : 8 * t + ky + 8, kx : kx + W],
                        start=(o == 0),
                        stop=(o == 8),
                        skip_group_check=True,
                    )

                def conv2_drain(cc, t, ps2):
                    sc = R.tile([128, 512], F32, tag="scr",
                                name=f"scr2_{cc}_{t}")
                    nc.scalar.activation(
                        out=h2[cc][:, 512 * t : 512 * (t + 1)],
                        in_=ps2[:],
                        func=AF.Copy,
                        accum_out=s2[:, cc * NT + t : cc * NT + t + 1],
                    )
                    nc.vector.scalar_tensor_tensor(
                        out=sc[:],
                        in0=ps2[:],
                        scalar=1.0,
                        in1=h2[cc][:, 512 * t : 512 * (t + 1)],
                        op0=ALU.mult,
                        op1=ALU.mult,
                        accum_out=q2[:, cc * NT + t : cc * NT + t + 1],
                    )

                def chunk_ar(cc):
                    nc.vector.reduce_sum(
                        st2[:, 2 * cc : 2 * cc + 1],
                        s2[:, cc * NT : (cc + 1) * NT], axis=AX.X,
                    )
                    nc.vector.reduce_sum(
                        st2[:, 2 * cc + 1 : 2 * cc + 2],
                        q2[:, cc * NT : (cc + 1) * NT], axis=AX.X,
                    )
                    nc.scalar.dma_start(
                        out=ar2i[cc][:], in_=st2[:, 2 * cc : 2 * cc + 2]
                    )
                    nc.gpsimd.collective_compute(
                        "AllReduce",
                        ALU.add,
                        replica_groups=[list(range(N_CORES))],
                        ins=[ar2i[cc].opt()],
                        outs=[ar2o[cc].opt()],
                    )
                    nc.gpsimd.dma_start(
                        out=st2g[:, 2 * cc : 2 * cc + 2], in_=ar2o[cc][:]
                    )

                for cc in range(NC2):
                    ps2t = [
                        PS.tile([128, 512], F32, tag="ps",
                                name=f"ps2_{cc}_{t}")
                        for t in range(NT)
                    ]
                    if cc < NC2 - 1:
                        # weight-stationary in two half-sets of 4 PSUM banks so
                        # the next chunk's first pass never waits on drains
                        for hf in range(2):
                            for o in range(9):
                                for t in range(4 * hf, 4 * hf + 4):
                                    conv2_mm(cc, t, o, ps2t[t])
                            for t in range(4 * hf, 4 * hf + 4):
                                conv2_drain(cc, t, ps2t[t])
                    else:
                        # last chunk tile-major: stats ready right at the end
                        for t in range(NT):
                            for o in range(9):
                                conv2_mm(cc, t, o, ps2t[t])
                            conv2_drain(cc, t, ps2t[t])
                    chunk_ar(cc)
                    if cc >= 2:
                        bn2_coefs(cc - 2)
                        bn2_apply(cc - 2)

                for cc in range(NC2 - 2, NC2):
                    bn2_coefs(cc)
                    bn2_apply(cc)
    nc.compile()
    return nc


_CACHE = {}


def _get_nc():
    if "nc" not in _CACHE:
        nc = bacc.Bacc("TRN2", target_bir_lowering=False, debug=False,
                       num_devices=N_CORES)
        _CACHE["nc"] = _emit(nc)
    return _CACHE["nc"]


def _prep_inputs(x, w1, gamma1, bnb1, beta, w2, gamma2, bnb2):
    x = np.asarray(x, np.float32)
    xp = np.zeros((B, CIN, HP, WP), np.float32)
    xp[:, :, 1 : 1 + H, 1 : 1 + W] = x
    xp = xp.reshape(B, CIN, PPAD).astype(BF16_NP)

    w1t = (
        np.asarray(w1, np.float32)
        .transpose(1, 2, 3, 0)                     # [cin, ky, kx, cout]
        .reshape(NC1, 128, 9, 128)                 # [c, p, o, m]
        .transpose(1, 0, 2, 3)                     # [p, c, o, m]
        .reshape(128, NC1 * 9 * 128)
        .astype(BF16_NP)
    )
    w2t = (
        np.asarray(w2, np.float32)
        .transpose(1, 2, 3, 0)                     # [cin=128, ky, kx, cout=512]
        .reshape(128, 9, NC2, 128)                 # [p, o, cc, m]
        .reshape(128, 9 * NC2 * 128)
        .astype(BF16_NP)
    )
    g1b1 = np.stack(
        [np.asarray(gamma1, np.float32), np.asarray(bnb1, np.float32)], axis=1
    )
    g2b2 = np.concatenate(
        [
            np.asarray(gamma2, np.float32).reshape(NC2, 128).T,
            np.asarray(bnb2, np.float32).reshape(NC2, 128).T,
        ],
        axis=1,
    )
    betav = np.full((128, 1), np.asarray(beta, np.float32)[0], np.float32)
    ident = np.eye(128, dtype=BF16_NP)

    shared = {
        "w1t": np.ascontiguousarray(w1t),
        "w2t": np.ascontiguousarray(w2t),
        "g1b1": np.ascontiguousarray(g1b1),
        "g2b2": np.ascontiguousarray(g2b2),
        "betav": betav,
        "ident": ident,
    }
    return [dict(shared, xp=np.ascontiguousarray(xp[b])) for b in range(B)]


def kernel_with_results(inputs, trace=False):
    in_maps = _prep_inputs(**inputs)
    nc = _get_nc()
    res = bass_utils.run_bass_kernel_spmd(
        nc, in_maps, core_ids=list(range(N_CORES)), trace=trace
    )
    outs = np.stack([res.results[b]["out"] for b in range(B)])
    return outs.reshape(B, CIN, H, W).astype(np.float32), res


def kernel(**inputs):
    out, _ = kernel_with_results(inputs, trace=False)
    return out
